# revision 9
# baseline (speedup 1.0000x reference)
"""PiInterferenceLayer Trainium2 kernel.

out[b, p] = |sum_d conj(x[b, d]) * (patterns[p, d] * e^{i*phase_p})|^2

Host folds the phases into the patterns:
  sr = patterns_real*cos(phi) - patterns_imag*sin(phi)   # [P, D]
  si = patterns_real*sin(phi) + patterns_imag*cos(phi)
  Zr = xr @ sr.T + xi @ si.T       # [B, P]
  Zi = xr @ si.T - xi @ sr.T
  out = Zr^2 + Zi^2

3-multiplication complex GEMM (Karatsuba / Gauss) with a = xr, b = -xi,
c = sr, d = si:
  m1 = xr @ sr.T
  m2p = xi @ si.T                      (= -m2)
  m3 = (xr - xi) @ (sr + si).T
  Zr = m1 + m2p
  Zi = m3 - m1 + m2p

Sharding: batch (8192) split 8 ways across NeuronCores, patterns replicated.
Device computes the TRANSPOSED output outT[p, b] per core (patterns are the
matmul stationary side, x is the moving side); host transposes back.

Per core (B_c = 1024):
  - moving x-side (xrT, xiT, (xr-xi)T as [D, B_c]) fully resident in SBUF
  - stationary pattern tiles ([D, P] arrays, 128-pattern columns) streamed,
    double buffered
  - TensorE fp32r (FP22) matmuls accumulate m1/m2p/m3 in PSUM over the
    contraction; VectorE combines + squares; DMA out.
"""

from contextlib import ExitStack

import numpy as np

import concourse.bass as bass
import concourse.mybir as mybir
import concourse.tile as tile
from concourse import bacc
from concourse.bass_utils import run_bass_kernel_spmd

B = 8192
D = 1024
P = 4096
NCORES = 8
BC = B // NCORES  # rows of x per core

KT = D // 128  # contraction tiles
NPT = P // 128  # pattern tiles (stationary, 128 wide)
BCH = 512  # moving-side batch chunk (one PSUM bank of fp32)
NBCH = BC // BCH

_prog_cache = {}


def build_program():
    if "nc" in _prog_cache:
        return _prog_cache["nc"]

    nc = bacc.Bacc(
        "TRN2",
        target_bir_lowering=False,
        debug=False,
        enable_asserts=False,
        num_devices=NCORES,
    )
    f32 = mybir.dt.float32
    f32r = mybir.dt.float32r

    xrT = nc.dram_tensor("xrT", [D, BC], f32, kind="ExternalInput").ap()
    xiT = nc.dram_tensor("xiT", [D, BC], f32, kind="ExternalInput").ap()
    wT = nc.dram_tensor("wT", [D, BC], f32, kind="ExternalInput").ap()
    srT = nc.dram_tensor("srT", [D, P], f32, kind="ExternalInput").ap()
    siT = nc.dram_tensor("siT", [D, P], f32, kind="ExternalInput").ap()
    ssT = nc.dram_tensor("ssT", [D, P], f32, kind="ExternalInput").ap()
    out = nc.dram_tensor("out", [P, BC], f32, kind="ExternalOutput").ap()

    # [D, P] viewed as [q=partition, k, p] for single-DMA stripe loads
    srT_v = srT.bitcast(f32r).rearrange("(k q) p -> q k p", q=128)
    siT_v = siT.bitcast(f32r).rearrange("(k q) p -> q k p", q=128)
    ssT_v = ssT.bitcast(f32r).rearrange("(k q) p -> q k p", q=128)

    with ExitStack() as ctx:
        tc = ctx.enter_context(tile.TileContext(nc))

        xpool = ctx.enter_context(tc.tile_pool(name="xres", bufs=1))
        spool = ctx.enter_context(tc.tile_pool(name="pat", bufs=3))
        ps1_pool = ctx.enter_context(tc.tile_pool(name="ps1", bufs=2, space="PSUM"))
        ps2_pool = ctx.enter_context(tc.tile_pool(name="ps2", bufs=2, space="PSUM"))
        ps3_pool = ctx.enter_context(tc.tile_pool(name="ps3", bufs=2, space="PSUM"))
        e_pool = ctx.enter_context(tc.tile_pool(name="epi", bufs=2))
        o_pool = ctx.enter_context(tc.tile_pool(name="osb", bufs=3))

        # Resident moving-side x: [128, KT, BC] each (32KB/partition), 3 arrays.
        # Loaded in [128, 512] chunks, batch-chunk-major, so the first
        # (p_tile, b_chunk) matmuls can start as soon as their k-tile lands.
        xr_sb = xpool.tile([128, KT, BC], f32r, tag="xr", name="xr_sb")
        xi_sb = xpool.tile([128, KT, BC], f32r, tag="xi", name="xi_sb")
        w_sb = xpool.tile([128, KT, BC], f32r, tag="w", name="w_sb")
        for bc in range(NBCH):
            bsl = slice(bc * BCH, (bc + 1) * BCH)
            for k in range(KT):
                ksl = slice(k * 128, (k + 1) * 128)
                nc.scalar.dma_start(xr_sb[:, k, bsl], xrT[ksl, bsl].bitcast(f32r))
                nc.scalar.dma_start(xi_sb[:, k, bsl], xiT[ksl, bsl].bitcast(f32r))
                nc.scalar.dma_start(w_sb[:, k, bsl], wT[ksl, bsl].bitcast(f32r))

        for pt in range(NPT):
            psl = slice(pt * 128, (pt + 1) * 128)
            sr_sb = spool.tile([128, KT, 128], f32r, tag="sr", name="sr_sb")
            si_sb = spool.tile([128, KT, 128], f32r, tag="si", name="si_sb")
            ss_sb = spool.tile([128, KT, 128], f32r, tag="ss", name="ss_sb")
            nc.sync.dma_start(sr_sb[:], srT_v[:, :, psl])
            nc.sync.dma_start(si_sb[:], siT_v[:, :, psl])
            nc.sync.dma_start(ss_sb[:], ssT_v[:, :, psl])

            for bc in range(NBCH):
                bsl = slice(bc * BCH, (bc + 1) * BCH)
                ps1 = ps1_pool.tile([128, BCH], f32, name="ps1")
                ps2 = ps2_pool.tile([128, BCH], f32, name="ps2")
                ps3 = ps3_pool.tile([128, BCH], f32, name="ps3")
                for k in range(KT):
                    first = k == 0
                    last = k == KT - 1
                    nc.tensor.matmul(
                        ps1[:], sr_sb[:, k, :], xr_sb[:, k, bsl], start=first, stop=last
                    )
                    nc.tensor.matmul(
                        ps2[:], si_sb[:, k, :], xi_sb[:, k, bsl], start=first, stop=last
                    )
                    nc.tensor.matmul(
                        ps3[:], ss_sb[:, k, :], w_sb[:, k, bsl], start=first, stop=last
                    )

                # Zr = m1 + m2p ; Zi = m3 - m1 + m2p ; out = Zr^2 + Zi^2
                # 6 VectorE ops; the final add rides the SWDGE store
                # (accum_op) to keep it off the DVE.
                c2 = e_pool.tile([128, BCH], f32, tag="c2", name="c2")
                nc.vector.tensor_copy(c2[:], ps2[:])
                zr = e_pool.tile([128, BCH], f32, tag="zr", name="zr")
                nc.vector.tensor_add(zr[:], c2[:], ps1[:])
                t3 = e_pool.tile([128, BCH], f32, tag="t3", name="t3")
                nc.vector.tensor_add(t3[:], c2[:], ps3[:])
                zi = e_pool.tile([128, BCH], f32, tag="zi", name="zi")
                nc.vector.tensor_sub(zi[:], t3[:], ps1[:])
                sq_r = o_pool.tile([128, BCH], f32, tag="sqr", name="sq_r")
                nc.vector.tensor_mul(sq_r[:], zr[:], zr[:])
                sq_i = e_pool.tile([128, BCH], f32, tag="sqi", name="sq_i")
                nc.vector.tensor_mul(sq_i[:], zi[:], zi[:])
                nc.gpsimd.dma_start(
                    sq_r[:], sq_i[:], accum_op=mybir.AluOpType.add
                )
                nc.gpsimd.dma_start(out[psl, bsl], sq_r[:])

    nc.compile()
    _prog_cache["nc"] = nc
    return nc


def _prepare_in_maps(x_real, x_imag, patterns_real, patterns_imag, phases):
    x_real = np.ascontiguousarray(np.asarray(x_real, dtype=np.float32))
    x_imag = np.ascontiguousarray(np.asarray(x_imag, dtype=np.float32))
    patterns_real = np.asarray(patterns_real, dtype=np.float32)
    patterns_imag = np.asarray(patterns_imag, dtype=np.float32)
    ph = np.asarray(phases, dtype=np.float64)

    cos = np.cos(ph)[:, None]
    sin = np.sin(ph)[:, None]
    pr = patterns_real.astype(np.float64)
    pi = patterns_imag.astype(np.float64)
    sr = (pr * cos - pi * sin).astype(np.float32)  # [P, D]
    si = (pr * sin + pi * cos).astype(np.float32)
    srT = np.ascontiguousarray(sr.T)  # [D, P]
    siT = np.ascontiguousarray(si.T)
    ssT = srT + siT

    in_maps = []
    for c in range(NCORES):
        rows = slice(c * BC, (c + 1) * BC)
        xs = x_real[rows]
        xis = x_imag[rows]
        in_maps.append(
            {
                "xrT": np.ascontiguousarray(xs.T),
                "xiT": np.ascontiguousarray(xis.T),
                "wT": np.ascontiguousarray(xs.T - xis.T),
                "srT": srT,
                "siT": siT,
                "ssT": ssT,
            }
        )
    return in_maps


def run(inputs, trace=False, **trace_kwargs):
    """Build + run on 8 cores. Returns (full_output, BassKernelResults)."""
    in_maps = _prepare_in_maps(**inputs)
    nc = build_program()
    res = run_bass_kernel_spmd(
        nc, in_maps, list(range(NCORES)), trace=trace, **trace_kwargs
    )
    full = np.concatenate(
        [np.ascontiguousarray(r["out"].T) for r in res.results], axis=0
    )
    return full, res


def kernel(x_real, x_imag, patterns_real, patterns_imag, phases):
    full, _ = run(
        {
            "x_real": x_real,
            "x_imag": x_imag,
            "patterns_real": patterns_real,
            "patterns_imag": patterns_imag,
            "phases": phases,
        }
    )
    return full


# revision 12
# speedup vs baseline: 1.1430x; 1.1430x over previous
"""PiInterferenceLayer Trainium2 kernel.

out[b, p] = |sum_d conj(x[b, d]) * (patterns[p, d] * e^{i*phase_p})|^2

Host folds the phases into the patterns:
  sr = patterns_real*cos(phi) - patterns_imag*sin(phi)   # [P, D]
  si = patterns_real*sin(phi) + patterns_imag*cos(phi)
  Zr = xr @ sr.T + xi @ si.T       # [B, P]
  Zi = xr @ si.T - xi @ sr.T
  out = Zr^2 + Zi^2

3-multiplication complex GEMM (Karatsuba / Gauss) with a = xr, b = -xi,
c = sr, d = si:
  m1 = xr @ sr.T
  m2p = xi @ si.T                      (= -m2)
  m3 = (xr - xi) @ (sr + si).T
  Zr = m1 + m2p
  Zi = m3 - m1 + m2p

Sharding: batch (8192) split 8 ways across NeuronCores, patterns replicated.
Device computes the TRANSPOSED output outT[p, b] per core (patterns are the
matmul stationary side, x is the moving side); host transposes back.

Per core (B_c = 1024):
  - moving x-side (xrT, xiT, (xr-xi)T as [D, B_c]) fully resident in SBUF
  - stationary pattern tiles ([D, P] arrays, 128-pattern columns) streamed,
    double buffered
  - TensorE fp32r (FP22) matmuls accumulate m1/m2p/m3 in PSUM over the
    contraction; VectorE combines + squares; DMA out.
"""

from contextlib import ExitStack

import numpy as np

import concourse.bass as bass
import concourse.mybir as mybir
import concourse.tile as tile
from concourse import bacc
from concourse.bass_utils import run_bass_kernel_spmd

B = 8192
D = 1024
P = 4096
NCORES = 8
BC = B // NCORES  # rows of x per core

KT = D // 128  # contraction tiles
NPT = P // 128  # pattern tiles (stationary, 128 wide)
BCH = 512  # moving-side batch chunk (one PSUM bank of fp32)
NBCH = BC // BCH

_prog_cache = {}


def build_program():
    if "nc" in _prog_cache:
        return _prog_cache["nc"]

    nc = bacc.Bacc(
        "TRN2",
        target_bir_lowering=False,
        debug=False,
        enable_asserts=False,
        num_devices=NCORES,
    )
    f32 = mybir.dt.float32
    f32r = mybir.dt.float32r

    xrT = nc.dram_tensor("xrT", [D, BC], f32, kind="ExternalInput").ap()
    xiT = nc.dram_tensor("xiT", [D, BC], f32, kind="ExternalInput").ap()
    wT = nc.dram_tensor("wT", [D, BC], f32, kind="ExternalInput").ap()
    srT = nc.dram_tensor("srT", [D, P], f32, kind="ExternalInput").ap()
    siT = nc.dram_tensor("siT", [D, P], f32, kind="ExternalInput").ap()
    ssT = nc.dram_tensor("ssT", [D, P], f32, kind="ExternalInput").ap()
    out = nc.dram_tensor("out", [P, BC], f32, kind="ExternalOutput").ap()

    # [D, P] viewed as [q=partition, k, p] for single-DMA stripe loads
    srT_v = srT.bitcast(f32r).rearrange("(k q) p -> q k p", q=128)
    siT_v = siT.bitcast(f32r).rearrange("(k q) p -> q k p", q=128)
    ssT_v = ssT.bitcast(f32r).rearrange("(k q) p -> q k p", q=128)

    with ExitStack() as ctx:
        tc = ctx.enter_context(tile.TileContext(nc))

        xpool = ctx.enter_context(tc.tile_pool(name="xres", bufs=1))
        spool = ctx.enter_context(tc.tile_pool(name="pat", bufs=3))
        ps1_pool = ctx.enter_context(tc.tile_pool(name="ps1", bufs=2, space="PSUM"))
        ps2_pool = ctx.enter_context(tc.tile_pool(name="ps2", bufs=2, space="PSUM"))
        ps3_pool = ctx.enter_context(tc.tile_pool(name="ps3", bufs=2, space="PSUM"))
        e_pool = ctx.enter_context(tc.tile_pool(name="epi", bufs=3))
        o_pool = ctx.enter_context(tc.tile_pool(name="osb", bufs=3))

        # Resident moving-side x: [128, KT, BC] each (32KB/partition), 3 arrays.
        # Loaded in [128, 512] chunks, batch-chunk-major, so the first
        # (p_tile, b_chunk) matmuls can start as soon as their k-tile lands.
        xr_sb = xpool.tile([128, KT, BC], f32r, tag="xr", name="xr_sb")
        xi_sb = xpool.tile([128, KT, BC], f32r, tag="xi", name="xi_sb")
        w_sb = xpool.tile([128, KT, BC], f32r, tag="w", name="w_sb")
        for bc in range(NBCH):
            bsl = slice(bc * BCH, (bc + 1) * BCH)
            for k in range(KT):
                ksl = slice(k * 128, (k + 1) * 128)
                nc.scalar.dma_start(xr_sb[:, k, bsl], xrT[ksl, bsl].bitcast(f32r))
                nc.scalar.dma_start(xi_sb[:, k, bsl], xiT[ksl, bsl].bitcast(f32r))
                nc.scalar.dma_start(w_sb[:, k, bsl], wT[ksl, bsl].bitcast(f32r))

        for pt in range(NPT):
            psl = slice(pt * 128, (pt + 1) * 128)
            sr_sb = spool.tile([128, KT, 128], f32r, tag="sr", name="sr_sb")
            si_sb = spool.tile([128, KT, 128], f32r, tag="si", name="si_sb")
            ss_sb = spool.tile([128, KT, 128], f32r, tag="ss", name="ss_sb")
            nc.sync.dma_start(sr_sb[:], srT_v[:, :, psl])
            nc.sync.dma_start(si_sb[:], siT_v[:, :, psl])
            nc.sync.dma_start(ss_sb[:], ssT_v[:, :, psl])

            for bc in range(NBCH):
                bsl = slice(bc * BCH, (bc + 1) * BCH)
                ps1 = ps1_pool.tile([128, BCH], f32, name="ps1")
                ps2 = ps2_pool.tile([128, BCH], f32, name="ps2")
                ps3 = ps3_pool.tile([128, BCH], f32, name="ps3")
                for k in range(KT):
                    first = k == 0
                    last = k == KT - 1
                    nc.tensor.matmul(
                        ps1[:], sr_sb[:, k, :], xr_sb[:, k, bsl], start=first, stop=last
                    )
                    nc.tensor.matmul(
                        ps2[:], si_sb[:, k, :], xi_sb[:, k, bsl], start=first, stop=last
                    )
                    nc.tensor.matmul(
                        ps3[:], ss_sb[:, k, :], w_sb[:, k, bsl], start=first, stop=last
                    )

                # Zr = m1 + m2p ; Zi = m3 - m1 + m2p ; out = Zr^2 + Zi^2
                c2 = e_pool.tile([128, BCH], f32, tag="c2", name="c2")
                nc.vector.tensor_copy(c2[:], ps2[:])
                zr = e_pool.tile([128, BCH], f32, tag="zr", name="zr")
                nc.vector.tensor_add(zr[:], c2[:], ps1[:])
                t3 = e_pool.tile([128, BCH], f32, tag="t3", name="t3")
                nc.vector.tensor_add(t3[:], c2[:], ps3[:])
                zi = e_pool.tile([128, BCH], f32, tag="zi", name="zi")
                nc.vector.tensor_sub(zi[:], t3[:], ps1[:])
                sq_r = e_pool.tile([128, BCH], f32, tag="sqr", name="sq_r")
                nc.vector.tensor_mul(sq_r[:], zr[:], zr[:])
                sq_i = e_pool.tile([128, BCH], f32, tag="sqi", name="sq_i")
                nc.vector.tensor_mul(sq_i[:], zi[:], zi[:])
                o_sb = o_pool.tile([128, BCH], f32, name="o_sb")
                nc.vector.tensor_add(o_sb[:], sq_r[:], sq_i[:])
                nc.gpsimd.dma_start(out[psl, bsl], o_sb[:])

    nc.compile()
    _prog_cache["nc"] = nc
    return nc


def _prepare_in_maps(x_real, x_imag, patterns_real, patterns_imag, phases):
    x_real = np.ascontiguousarray(np.asarray(x_real, dtype=np.float32))
    x_imag = np.ascontiguousarray(np.asarray(x_imag, dtype=np.float32))
    patterns_real = np.asarray(patterns_real, dtype=np.float32)
    patterns_imag = np.asarray(patterns_imag, dtype=np.float32)
    ph = np.asarray(phases, dtype=np.float64)

    cos = np.cos(ph)[:, None]
    sin = np.sin(ph)[:, None]
    pr = patterns_real.astype(np.float64)
    pi = patterns_imag.astype(np.float64)
    sr = (pr * cos - pi * sin).astype(np.float32)  # [P, D]
    si = (pr * sin + pi * cos).astype(np.float32)
    srT = np.ascontiguousarray(sr.T)  # [D, P]
    siT = np.ascontiguousarray(si.T)
    ssT = srT + siT

    in_maps = []
    for c in range(NCORES):
        rows = slice(c * BC, (c + 1) * BC)
        xs = x_real[rows]
        xis = x_imag[rows]
        in_maps.append(
            {
                "xrT": np.ascontiguousarray(xs.T),
                "xiT": np.ascontiguousarray(xis.T),
                "wT": np.ascontiguousarray(xs.T - xis.T),
                "srT": srT,
                "siT": siT,
                "ssT": ssT,
            }
        )
    return in_maps


def run(inputs, trace=False, **trace_kwargs):
    """Build + run on 8 cores. Returns (full_output, BassKernelResults)."""
    in_maps = _prepare_in_maps(**inputs)
    nc = build_program()
    try:
        res = run_bass_kernel_spmd(
            nc, in_maps, list(range(NCORES)), trace=trace, **trace_kwargs
        )
    except Exception:
        # One retry: transient NRT device errors recover on re-execution.
        import time as _time

        _time.sleep(10)
        res = run_bass_kernel_spmd(
            nc, in_maps, list(range(NCORES)), trace=trace, **trace_kwargs
        )
    full = np.concatenate(
        [np.ascontiguousarray(r["out"].T) for r in res.results], axis=0
    )
    return full, res


def kernel(x_real, x_imag, patterns_real, patterns_imag, phases):
    full, _ = run(
        {
            "x_real": x_real,
            "x_imag": x_imag,
            "patterns_real": patterns_real,
            "patterns_imag": patterns_imag,
            "phases": phases,
        }
    )
    return full


# revision 14
# speedup vs baseline: 1.1809x; 1.0332x over previous
"""PiInterferenceLayer Trainium2 kernel.

out[b, p] = |sum_d conj(x[b, d]) * (patterns[p, d] * e^{i*phase_p})|^2

Host folds the phases into the patterns:
  sr = patterns_real*cos(phi) - patterns_imag*sin(phi)   # [P, D]
  si = patterns_real*sin(phi) + patterns_imag*cos(phi)
  Zr = xr @ sr.T + xi @ si.T       # [B, P]
  Zi = xr @ si.T - xi @ sr.T
  out = Zr^2 + Zi^2

3-multiplication complex GEMM (Karatsuba / Gauss) with a = xr, b = -xi,
c = sr, d = si:
  m1 = xr @ sr.T
  m2p = xi @ si.T                      (= -m2)
  m3 = (xr - xi) @ (sr + si).T
  Zr = m1 + m2p
  Zi = m3 - m1 + m2p

Sharding: batch (8192) split 8 ways across NeuronCores, patterns replicated.
Device computes the TRANSPOSED output outT[p, b] per core (patterns are the
matmul stationary side, x is the moving side); host transposes back.

Per core (B_c = 1024):
  - moving x-side (xrT, xiT, (xr-xi)T as [D, B_c]) fully resident in SBUF
  - stationary pattern tiles ([D, P] arrays, 128-pattern columns) streamed,
    double buffered
  - TensorE fp32r (FP22) matmuls accumulate m1/m2p/m3 in PSUM over the
    contraction; VectorE combines + squares; DMA out.
"""

from contextlib import ExitStack

import numpy as np

import concourse.bass as bass
import concourse.mybir as mybir
import concourse.tile as tile
from concourse import bacc
from concourse.bass_utils import run_bass_kernel_spmd

B = 8192
D = 1024
P = 4096
NCORES = 8
BC = B // NCORES  # rows of x per core

KT = D // 128  # contraction tiles
NPT = P // 128  # pattern tiles (stationary, 128 wide)
BCH = 512  # moving-side batch chunk (one PSUM bank of fp32)
NBCH = BC // BCH

_prog_cache = {}


def build_program():
    if "nc" in _prog_cache:
        return _prog_cache["nc"]

    nc = bacc.Bacc(
        "TRN2",
        target_bir_lowering=False,
        debug=False,
        enable_asserts=False,
        num_devices=NCORES,
    )
    f32 = mybir.dt.float32
    f32r = mybir.dt.float32r

    xrT = nc.dram_tensor("xrT", [D, BC], f32, kind="ExternalInput").ap()
    xiT = nc.dram_tensor("xiT", [D, BC], f32, kind="ExternalInput").ap()
    wT = nc.dram_tensor("wT", [D, BC], f32, kind="ExternalInput").ap()
    srT = nc.dram_tensor("srT", [D, P], f32, kind="ExternalInput").ap()
    siT = nc.dram_tensor("siT", [D, P], f32, kind="ExternalInput").ap()
    ssT = nc.dram_tensor("ssT", [D, P], f32, kind="ExternalInput").ap()
    out = nc.dram_tensor("out", [P, BC], f32, kind="ExternalOutput").ap()

    # [D, P] viewed as [q=partition, k, p] for single-DMA stripe loads
    srT_v = srT.bitcast(f32r).rearrange("(k q) p -> q k p", q=128)
    siT_v = siT.bitcast(f32r).rearrange("(k q) p -> q k p", q=128)
    ssT_v = ssT.bitcast(f32r).rearrange("(k q) p -> q k p", q=128)

    with ExitStack() as ctx:
        tc = ctx.enter_context(tile.TileContext(nc))

        xpool = ctx.enter_context(tc.tile_pool(name="xres", bufs=1))
        spool = ctx.enter_context(tc.tile_pool(name="pat", bufs=5))
        ps1_pool = ctx.enter_context(tc.tile_pool(name="ps1", bufs=2, space="PSUM"))
        ps2_pool = ctx.enter_context(tc.tile_pool(name="ps2", bufs=2, space="PSUM"))
        ps3_pool = ctx.enter_context(tc.tile_pool(name="ps3", bufs=2, space="PSUM"))
        e_pool = ctx.enter_context(tc.tile_pool(name="epi", bufs=2))
        o_pool = ctx.enter_context(tc.tile_pool(name="osb", bufs=3))

        def load_pattern_tiles(pt):
            psl = slice(pt * 128, (pt + 1) * 128)
            sr_sb = spool.tile([128, KT, 128], f32r, tag="sr", name="sr_sb")
            si_sb = spool.tile([128, KT, 128], f32r, tag="si", name="si_sb")
            ss_sb = spool.tile([128, KT, 128], f32r, tag="ss", name="ss_sb")
            nc.sync.dma_start(sr_sb[:], srT_v[:, :, psl])
            nc.sync.dma_start(si_sb[:], siT_v[:, :, psl])
            nc.sync.dma_start(ss_sb[:], ssT_v[:, :, psl])
            return sr_sb, si_sb, ss_sb

        # Preload the first PRE p_tiles' patterns; their b-chunk-0 iterations
        # run while the second half of x is still streaming in.
        PRE = 4
        pat_tiles = {}
        for pt in range(PRE):
            pat_tiles[pt] = load_pattern_tiles(pt)

        # Resident moving-side x: [128, KT, BC] each (32KB/partition), 3 arrays.
        # Loaded in [128, 512] chunks, batch-chunk-major, alternating between
        # two HWDGE queues, so the first (p_tile, b_chunk) matmuls can start
        # as soon as their k-tile lands.
        xr_sb = xpool.tile([128, KT, BC], f32r, tag="xr", name="xr_sb")
        xi_sb = xpool.tile([128, KT, BC], f32r, tag="xi", name="xi_sb")
        w_sb = xpool.tile([128, KT, BC], f32r, tag="w", name="w_sb")
        for bc in range(NBCH):
            bsl = slice(bc * BCH, (bc + 1) * BCH)
            for k in range(KT):
                ksl = slice(k * 128, (k + 1) * 128)
                eng = nc.scalar if k % 2 == 0 else nc.sync
                eng.dma_start(xr_sb[:, k, bsl], xrT[ksl, bsl].bitcast(f32r))
                eng.dma_start(xi_sb[:, k, bsl], xiT[ksl, bsl].bitcast(f32r))
                eng.dma_start(w_sb[:, k, bsl], wT[ksl, bsl].bitcast(f32r))

        # Iteration order: b-chunk 0 of the preloaded p_tiles first (covers
        # the window while b-chunk 1 of x streams), then their b-chunk 1,
        # then the rest in (pt, bc) order.
        schedule = [(pt, 0) for pt in range(PRE)]
        schedule += [(pt, 1) for pt in range(PRE)]
        for pt in range(PRE, NPT):
            schedule += [(pt, 0), (pt, 1)]

        for pt, bc in schedule:
            psl = slice(pt * 128, (pt + 1) * 128)
            if pt not in pat_tiles:
                pat_tiles[pt] = load_pattern_tiles(pt)
            sr_sb, si_sb, ss_sb = pat_tiles[pt]

            if True:
                bsl = slice(bc * BCH, (bc + 1) * BCH)
                ps1 = ps1_pool.tile([128, BCH], f32, name="ps1")
                ps2 = ps2_pool.tile([128, BCH], f32, name="ps2")
                ps3 = ps3_pool.tile([128, BCH], f32, name="ps3")
                for k in range(KT):
                    first = k == 0
                    last = k == KT - 1
                    nc.tensor.matmul(
                        ps1[:], sr_sb[:, k, :], xr_sb[:, k, bsl], start=first, stop=last
                    )
                    nc.tensor.matmul(
                        ps2[:], si_sb[:, k, :], xi_sb[:, k, bsl], start=first, stop=last
                    )
                    nc.tensor.matmul(
                        ps3[:], ss_sb[:, k, :], w_sb[:, k, bsl], start=first, stop=last
                    )

                # Zr = m1 + m2p ; Zi = m3 - m1 + m2p ; out = Zr^2 + Zi^2
                c2 = e_pool.tile([128, BCH], f32, tag="c2", name="c2")
                nc.vector.tensor_copy(c2[:], ps2[:])
                zr = e_pool.tile([128, BCH], f32, tag="zr", name="zr")
                nc.vector.tensor_add(zr[:], c2[:], ps1[:])
                t3 = e_pool.tile([128, BCH], f32, tag="t3", name="t3")
                nc.vector.tensor_add(t3[:], c2[:], ps3[:])
                zi = e_pool.tile([128, BCH], f32, tag="zi", name="zi")
                nc.vector.tensor_sub(zi[:], t3[:], ps1[:])
                sq_r = e_pool.tile([128, BCH], f32, tag="sqr", name="sq_r")
                nc.vector.tensor_mul(sq_r[:], zr[:], zr[:])
                sq_i = e_pool.tile([128, BCH], f32, tag="sqi", name="sq_i")
                nc.vector.tensor_mul(sq_i[:], zi[:], zi[:])
                o_sb = o_pool.tile([128, BCH], f32, name="o_sb")
                nc.vector.tensor_add(o_sb[:], sq_r[:], sq_i[:])
                nc.gpsimd.dma_start(out[psl, bsl], o_sb[:])

    nc.compile()
    _prog_cache["nc"] = nc
    return nc


def _prepare_in_maps(x_real, x_imag, patterns_real, patterns_imag, phases):
    x_real = np.ascontiguousarray(np.asarray(x_real, dtype=np.float32))
    x_imag = np.ascontiguousarray(np.asarray(x_imag, dtype=np.float32))
    patterns_real = np.asarray(patterns_real, dtype=np.float32)
    patterns_imag = np.asarray(patterns_imag, dtype=np.float32)
    ph = np.asarray(phases, dtype=np.float64)

    cos = np.cos(ph)[:, None]
    sin = np.sin(ph)[:, None]
    pr = patterns_real.astype(np.float64)
    pi = patterns_imag.astype(np.float64)
    sr = (pr * cos - pi * sin).astype(np.float32)  # [P, D]
    si = (pr * sin + pi * cos).astype(np.float32)
    srT = np.ascontiguousarray(sr.T)  # [D, P]
    siT = np.ascontiguousarray(si.T)
    ssT = srT + siT

    in_maps = []
    for c in range(NCORES):
        rows = slice(c * BC, (c + 1) * BC)
        xs = x_real[rows]
        xis = x_imag[rows]
        in_maps.append(
            {
                "xrT": np.ascontiguousarray(xs.T),
                "xiT": np.ascontiguousarray(xis.T),
                "wT": np.ascontiguousarray(xs.T - xis.T),
                "srT": srT,
                "siT": siT,
                "ssT": ssT,
            }
        )
    return in_maps


def run(inputs, trace=False, **trace_kwargs):
    """Build + run on 8 cores. Returns (full_output, BassKernelResults)."""
    in_maps = _prepare_in_maps(**inputs)
    nc = build_program()
    try:
        res = run_bass_kernel_spmd(
            nc, in_maps, list(range(NCORES)), trace=trace, **trace_kwargs
        )
    except Exception:
        # One retry: transient NRT device errors recover on re-execution.
        import time as _time

        _time.sleep(10)
        res = run_bass_kernel_spmd(
            nc, in_maps, list(range(NCORES)), trace=trace, **trace_kwargs
        )
    full = np.concatenate(
        [np.ascontiguousarray(r["out"].T) for r in res.results], axis=0
    )
    return full, res


def kernel(x_real, x_imag, patterns_real, patterns_imag, phases):
    full, _ = run(
        {
            "x_real": x_real,
            "x_imag": x_imag,
            "patterns_real": patterns_real,
            "patterns_imag": patterns_imag,
            "phases": phases,
        }
    )
    return full


# revision 17
# speedup vs baseline: 1.1940x; 1.0111x over previous
"""PiInterferenceLayer Trainium2 kernel.

out[b, p] = |sum_d conj(x[b, d]) * (patterns[p, d] * e^{i*phase_p})|^2

Host folds the phases into the patterns:
  sr = patterns_real*cos(phi) - patterns_imag*sin(phi)   # [P, D]
  si = patterns_real*sin(phi) + patterns_imag*cos(phi)
  Zr = xr @ sr.T + xi @ si.T       # [B, P]
  Zi = xr @ si.T - xi @ sr.T
  out = Zr^2 + Zi^2

3-multiplication complex GEMM (Karatsuba / Gauss) with a = xr, b = -xi,
c = sr, d = si:
  m1 = xr @ sr.T
  m2p = xi @ si.T                      (= -m2)
  m3 = (xr - xi) @ (sr + si).T
  Zr = m1 + m2p
  Zi = m3 - m1 + m2p

Sharding: batch (8192) split 8 ways across NeuronCores, patterns replicated.
Device computes the TRANSPOSED output outT[p, b] per core (patterns are the
matmul stationary side, x is the moving side); host transposes back.

Per core (B_c = 1024):
  - moving x-side (xrT, xiT, (xr-xi)T as [D, B_c]) fully resident in SBUF
  - stationary pattern tiles ([D, P] arrays, 128-pattern columns) streamed,
    double buffered
  - TensorE fp32r (FP22) matmuls accumulate m1/m2p/m3 in PSUM over the
    contraction; VectorE combines + squares; DMA out.
"""

from contextlib import ExitStack

import numpy as np

import concourse.bass as bass
import concourse.mybir as mybir
import concourse.tile as tile
from concourse import bacc
from concourse.bass_utils import run_bass_kernel_spmd

B = 8192
D = 1024
P = 4096
NCORES = 8
BC = B // NCORES  # rows of x per core

KT = D // 128  # contraction tiles
NPT = P // 128  # pattern tiles (stationary, 128 wide)
BCH = 512  # moving-side batch chunk (one PSUM bank of fp32)
NBCH = BC // BCH

_prog_cache = {}


def build_program():
    if "nc" in _prog_cache:
        return _prog_cache["nc"]

    nc = bacc.Bacc(
        "TRN2",
        target_bir_lowering=False,
        debug=False,
        enable_asserts=False,
        num_devices=NCORES,
    )
    f32 = mybir.dt.float32
    f32r = mybir.dt.float32r

    xrT = nc.dram_tensor("xrT", [D, BC], f32, kind="ExternalInput").ap()
    xiT = nc.dram_tensor("xiT", [D, BC], f32, kind="ExternalInput").ap()
    srT = nc.dram_tensor("srT", [D, P], f32, kind="ExternalInput").ap()
    siT = nc.dram_tensor("siT", [D, P], f32, kind="ExternalInput").ap()
    ssT = nc.dram_tensor("ssT", [D, P], f32, kind="ExternalInput").ap()
    out = nc.dram_tensor("out", [P, BC], f32, kind="ExternalOutput").ap()

    # [D, P] viewed as [q=partition, k, p] for single-DMA stripe loads
    srT_v = srT.bitcast(f32r).rearrange("(k q) p -> q k p", q=128)
    siT_v = siT.bitcast(f32r).rearrange("(k q) p -> q k p", q=128)
    ssT_v = ssT.bitcast(f32r).rearrange("(k q) p -> q k p", q=128)

    with ExitStack() as ctx:
        tc = ctx.enter_context(tile.TileContext(nc))

        xpool = ctx.enter_context(tc.tile_pool(name="xres", bufs=1))
        spool = ctx.enter_context(tc.tile_pool(name="pat", bufs=5))
        ps1_pool = ctx.enter_context(tc.tile_pool(name="ps1", bufs=2, space="PSUM"))
        ps2_pool = ctx.enter_context(tc.tile_pool(name="ps2", bufs=2, space="PSUM"))
        ps3_pool = ctx.enter_context(tc.tile_pool(name="ps3", bufs=2, space="PSUM"))
        e_pool = ctx.enter_context(tc.tile_pool(name="epi", bufs=2))
        o_pool = ctx.enter_context(tc.tile_pool(name="osb", bufs=3))

        def load_pattern_tiles(pt):
            psl = slice(pt * 128, (pt + 1) * 128)
            sr_sb = spool.tile([128, KT, 128], f32r, tag="sr", name="sr_sb")
            si_sb = spool.tile([128, KT, 128], f32r, tag="si", name="si_sb")
            ss_sb = spool.tile([128, KT, 128], f32r, tag="ss", name="ss_sb")
            nc.sync.dma_start(sr_sb[:], srT_v[:, :, psl])
            nc.sync.dma_start(si_sb[:], siT_v[:, :, psl])
            nc.sync.dma_start(ss_sb[:], ssT_v[:, :, psl])
            return sr_sb, si_sb, ss_sb

        # Preload the first PRE p_tiles' patterns; their b-chunk-0 iterations
        # run while the second half of x is still streaming in.
        PRE = 4
        pat_tiles = {}
        for pt in range(PRE):
            pat_tiles[pt] = load_pattern_tiles(pt)

        # Resident moving-side x: [128, KT, BC] each (32KB/partition), 3 arrays.
        # Loaded in [128, 512] chunks, batch-chunk-major, alternating between
        # two HWDGE queues, so the first (p_tile, b_chunk) matmuls can start
        # as soon as their k-tile lands.
        xr_sb = xpool.tile([128, KT, BC], f32r, tag="xr", name="xr_sb")
        xi_sb = xpool.tile([128, KT, BC], f32r, tag="xi", name="xi_sb")
        w_sb = xpool.tile([128, KT, BC], f32r, tag="w", name="w_sb")
        for bc in range(NBCH):
            bsl = slice(bc * BCH, (bc + 1) * BCH)
            for k in range(KT):
                ksl = slice(k * 128, (k + 1) * 128)
                eng = nc.scalar if k % 2 == 0 else nc.sync
                eng.dma_start(xr_sb[:, k, bsl], xrT[ksl, bsl].bitcast(f32r))
                eng.dma_start(xi_sb[:, k, bsl], xiT[ksl, bsl].bitcast(f32r))
                # w = xr - xi computed on-device (VectorE is idle during the
                # load phase) to cut startup HBM traffic by a third.
                nc.vector.tensor_sub(
                    w_sb[:, k, bsl], xr_sb[:, k, bsl], xi_sb[:, k, bsl]
                )

        # Iteration order: b-chunk 0 of the preloaded p_tiles first (covers
        # the window while b-chunk 1 of x streams), then their b-chunk 1,
        # then the rest in (pt, bc) order.
        schedule = [(pt, 0) for pt in range(PRE)]
        schedule += [(pt, 1) for pt in range(PRE)]
        for pt in range(PRE, NPT):
            schedule += [(pt, 0), (pt, 1)]

        for pt, bc in schedule:
            psl = slice(pt * 128, (pt + 1) * 128)
            if pt not in pat_tiles:
                pat_tiles[pt] = load_pattern_tiles(pt)
            sr_sb, si_sb, ss_sb = pat_tiles[pt]

            if True:
                bsl = slice(bc * BCH, (bc + 1) * BCH)
                ps1 = ps1_pool.tile([128, BCH], f32, name="ps1")
                ps2 = ps2_pool.tile([128, BCH], f32, name="ps2")
                ps3 = ps3_pool.tile([128, BCH], f32, name="ps3")
                for k in range(KT):
                    first = k == 0
                    last = k == KT - 1
                    nc.tensor.matmul(
                        ps1[:], sr_sb[:, k, :], xr_sb[:, k, bsl], start=first, stop=last
                    )
                    nc.tensor.matmul(
                        ps2[:], si_sb[:, k, :], xi_sb[:, k, bsl], start=first, stop=last
                    )
                    nc.tensor.matmul(
                        ps3[:], ss_sb[:, k, :], w_sb[:, k, bsl], start=first, stop=last
                    )

                # Zr = m1 + m2p ; Zi = m3 - m1 + m2p ; out = Zr^2 + Zi^2
                c2 = e_pool.tile([128, BCH], f32, tag="c2", name="c2")
                nc.vector.tensor_copy(c2[:], ps2[:])
                zr = e_pool.tile([128, BCH], f32, tag="zr", name="zr")
                nc.vector.tensor_add(zr[:], c2[:], ps1[:])
                t3 = e_pool.tile([128, BCH], f32, tag="t3", name="t3")
                nc.vector.tensor_add(t3[:], c2[:], ps3[:])
                zi = e_pool.tile([128, BCH], f32, tag="zi", name="zi")
                nc.vector.tensor_sub(zi[:], t3[:], ps1[:])
                sq_r = e_pool.tile([128, BCH], f32, tag="sqr", name="sq_r")
                nc.vector.tensor_mul(sq_r[:], zr[:], zr[:])
                sq_i = e_pool.tile([128, BCH], f32, tag="sqi", name="sq_i")
                nc.vector.tensor_mul(sq_i[:], zi[:], zi[:])
                o_sb = o_pool.tile([128, BCH], f32, name="o_sb")
                nc.vector.tensor_add(o_sb[:], sq_r[:], sq_i[:])
                nc.gpsimd.dma_start(out[psl, bsl], o_sb[:])

    nc.compile()
    _prog_cache["nc"] = nc
    return nc


def _prepare_in_maps(x_real, x_imag, patterns_real, patterns_imag, phases):
    x_real = np.ascontiguousarray(np.asarray(x_real, dtype=np.float32))
    x_imag = np.ascontiguousarray(np.asarray(x_imag, dtype=np.float32))
    patterns_real = np.asarray(patterns_real, dtype=np.float32)
    patterns_imag = np.asarray(patterns_imag, dtype=np.float32)
    ph = np.asarray(phases, dtype=np.float64)

    cos = np.cos(ph)[:, None]
    sin = np.sin(ph)[:, None]
    pr = patterns_real.astype(np.float64)
    pi = patterns_imag.astype(np.float64)
    sr = (pr * cos - pi * sin).astype(np.float32)  # [P, D]
    si = (pr * sin + pi * cos).astype(np.float32)
    srT = np.ascontiguousarray(sr.T)  # [D, P]
    siT = np.ascontiguousarray(si.T)
    ssT = srT + siT

    in_maps = []
    for c in range(NCORES):
        rows = slice(c * BC, (c + 1) * BC)
        xs = x_real[rows]
        xis = x_imag[rows]
        in_maps.append(
            {
                "xrT": np.ascontiguousarray(xs.T),
                "xiT": np.ascontiguousarray(xis.T),
                "srT": srT,
                "siT": siT,
                "ssT": ssT,
            }
        )
    return in_maps


def run(inputs, trace=False, **trace_kwargs):
    """Build + run on 8 cores. Returns (full_output, BassKernelResults)."""
    in_maps = _prepare_in_maps(**inputs)
    nc = build_program()
    try:
        res = run_bass_kernel_spmd(
            nc, in_maps, list(range(NCORES)), trace=trace, **trace_kwargs
        )
    except Exception:
        # One retry: transient NRT device errors recover on re-execution.
        import time as _time

        _time.sleep(10)
        res = run_bass_kernel_spmd(
            nc, in_maps, list(range(NCORES)), trace=trace, **trace_kwargs
        )
    full = np.concatenate(
        [np.ascontiguousarray(r["out"].T) for r in res.results], axis=0
    )
    return full, res


def kernel(x_real, x_imag, patterns_real, patterns_imag, phases):
    full, _ = run(
        {
            "x_real": x_real,
            "x_imag": x_imag,
            "patterns_real": patterns_real,
            "patterns_imag": patterns_imag,
            "phases": phases,
        }
    )
    return full


# revision 18
# speedup vs baseline: 1.2300x; 1.0302x over previous
"""PiInterferenceLayer Trainium2 kernel.

out[b, p] = |sum_d conj(x[b, d]) * (patterns[p, d] * e^{i*phase_p})|^2

Host folds the phases into the patterns:
  sr = patterns_real*cos(phi) - patterns_imag*sin(phi)   # [P, D]
  si = patterns_real*sin(phi) + patterns_imag*cos(phi)
  Zr = xr @ sr.T + xi @ si.T       # [B, P]
  Zi = xr @ si.T - xi @ sr.T
  out = Zr^2 + Zi^2

3-multiplication complex GEMM (Karatsuba / Gauss) with a = xr, b = -xi,
c = sr, d = si:
  m1 = xr @ sr.T
  m2p = xi @ si.T                      (= -m2)
  m3 = (xr - xi) @ (sr + si).T
  Zr = m1 + m2p
  Zi = m3 - m1 + m2p

Sharding: batch (8192) split 8 ways across NeuronCores, patterns replicated.
Device computes the TRANSPOSED output outT[p, b] per core (patterns are the
matmul stationary side, x is the moving side); host transposes back.

Per core (B_c = 1024):
  - moving x-side (xrT, xiT, (xr-xi)T as [D, B_c]) fully resident in SBUF
  - stationary pattern tiles ([D, P] arrays, 128-pattern columns) streamed,
    double buffered
  - TensorE fp32r (FP22) matmuls accumulate m1/m2p/m3 in PSUM over the
    contraction; VectorE combines + squares; DMA out.
"""

from contextlib import ExitStack

import numpy as np

import concourse.bass as bass
import concourse.mybir as mybir
import concourse.tile as tile
from concourse import bacc
from concourse.bass_utils import run_bass_kernel_spmd

B = 8192
D = 1024
P = 4096
NCORES = 8
BC = B // NCORES  # rows of x per core

KT = D // 128  # contraction tiles
NPT = P // 128  # pattern tiles (stationary, 128 wide)
BCH = 512  # moving-side batch chunk (one PSUM bank of fp32)
NBCH = BC // BCH

_prog_cache = {}


def build_program():
    if "nc" in _prog_cache:
        return _prog_cache["nc"]

    nc = bacc.Bacc(
        "TRN2",
        target_bir_lowering=False,
        debug=False,
        enable_asserts=False,
        num_devices=NCORES,
    )
    f32 = mybir.dt.float32
    f32r = mybir.dt.float32r

    xrT = nc.dram_tensor("xrT", [D, BC], f32, kind="ExternalInput").ap()
    xiT = nc.dram_tensor("xiT", [D, BC], f32, kind="ExternalInput").ap()
    srT = nc.dram_tensor("srT", [D, P], f32, kind="ExternalInput").ap()
    siT = nc.dram_tensor("siT", [D, P], f32, kind="ExternalInput").ap()
    ssT = nc.dram_tensor("ssT", [D, P], f32, kind="ExternalInput").ap()
    out = nc.dram_tensor("out", [P, BC], f32, kind="ExternalOutput").ap()

    # [D, P] viewed as [q=partition, k, p] for single-DMA stripe loads
    srT_v = srT.bitcast(f32r).rearrange("(k q) p -> q k p", q=128)
    siT_v = siT.bitcast(f32r).rearrange("(k q) p -> q k p", q=128)
    ssT_v = ssT.bitcast(f32r).rearrange("(k q) p -> q k p", q=128)

    with ExitStack() as ctx:
        tc = ctx.enter_context(tile.TileContext(nc))

        xpool = ctx.enter_context(tc.tile_pool(name="xres", bufs=1))
        spool = ctx.enter_context(tc.tile_pool(name="pat", bufs=5))
        ps1_pool = ctx.enter_context(tc.tile_pool(name="ps1", bufs=2, space="PSUM"))
        ps2_pool = ctx.enter_context(tc.tile_pool(name="ps2", bufs=2, space="PSUM"))
        ps3_pool = ctx.enter_context(tc.tile_pool(name="ps3", bufs=2, space="PSUM"))
        e_pool = ctx.enter_context(tc.tile_pool(name="epi", bufs=2))
        o_pool = ctx.enter_context(tc.tile_pool(name="osb", bufs=3))

        def load_pattern_tiles(pt):
            psl = slice(pt * 128, (pt + 1) * 128)
            sr_sb = spool.tile([128, KT, 128], f32r, tag="sr", name="sr_sb")
            si_sb = spool.tile([128, KT, 128], f32r, tag="si", name="si_sb")
            ss_sb = spool.tile([128, KT, 128], f32r, tag="ss", name="ss_sb")
            nc.sync.dma_start(sr_sb[:], srT_v[:, :, psl])
            nc.sync.dma_start(si_sb[:], siT_v[:, :, psl])
            nc.sync.dma_start(ss_sb[:], ssT_v[:, :, psl])
            return sr_sb, si_sb, ss_sb

        # Resident moving-side x: [128, KT, BC] each (32KB/partition), 3 arrays.
        # Loaded in [128, 512] chunks, batch-chunk-major, alternating between
        # two HWDGE queues, so the first (p_tile, b_chunk) matmuls can start
        # as soon as their k-tile lands. w = xr - xi is computed on-device
        # (VectorE is idle during the load phase) to cut startup HBM traffic.
        xr_sb = xpool.tile([128, KT, BC], f32r, tag="xr", name="xr_sb")
        xi_sb = xpool.tile([128, KT, BC], f32r, tag="xi", name="xi_sb")
        w_sb = xpool.tile([128, KT, BC], f32r, tag="w", name="w_sb")

        def load_x_chunk(bc, k):
            bsl = slice(bc * BCH, (bc + 1) * BCH)
            ksl = slice(k * 128, (k + 1) * 128)
            eng = nc.scalar if k % 2 == 0 else nc.sync
            eng.dma_start(xr_sb[:, k, bsl], xrT[ksl, bsl].bitcast(f32r))
            eng.dma_start(xi_sb[:, k, bsl], xiT[ksl, bsl].bitcast(f32r))
            nc.vector.tensor_sub(w_sb[:, k, bsl], xr_sb[:, k, bsl], xi_sb[:, k, bsl])

        # Emission (= per-queue DMA) order tuned so the sync queue serves
        # p_tile 0's stripes, then x's odd-k b0 chunks, then the remaining
        # preloaded stripes — the first matmul group is never stuck behind
        # pattern preloads.
        PRE = 4
        pat_tiles = {}
        pat_tiles[0] = load_pattern_tiles(0)
        for k in range(KT):
            load_x_chunk(0, k)
        for pt in range(1, PRE):
            pat_tiles[pt] = load_pattern_tiles(pt)
        for k in range(KT):
            load_x_chunk(1, k)

        # Iteration order: b-chunk 0 of the preloaded p_tiles first (covers
        # the window while b-chunk 1 of x streams), then their b-chunk 1,
        # then the rest in (pt, bc) order.
        schedule = [(pt, 0) for pt in range(PRE)]
        schedule += [(pt, 1) for pt in range(PRE)]
        for pt in range(PRE, NPT):
            schedule += [(pt, 0), (pt, 1)]

        for pt, bc in schedule:
            psl = slice(pt * 128, (pt + 1) * 128)
            if pt not in pat_tiles:
                pat_tiles[pt] = load_pattern_tiles(pt)
            sr_sb, si_sb, ss_sb = pat_tiles[pt]

            if True:
                bsl = slice(bc * BCH, (bc + 1) * BCH)
                ps1 = ps1_pool.tile([128, BCH], f32, name="ps1")
                ps2 = ps2_pool.tile([128, BCH], f32, name="ps2")
                ps3 = ps3_pool.tile([128, BCH], f32, name="ps3")
                for k in range(KT):
                    first = k == 0
                    last = k == KT - 1
                    nc.tensor.matmul(
                        ps1[:], sr_sb[:, k, :], xr_sb[:, k, bsl], start=first, stop=last
                    )
                    nc.tensor.matmul(
                        ps2[:], si_sb[:, k, :], xi_sb[:, k, bsl], start=first, stop=last
                    )
                    nc.tensor.matmul(
                        ps3[:], ss_sb[:, k, :], w_sb[:, k, bsl], start=first, stop=last
                    )

                # Zr = m1 + m2p ; Zi = m3 - m1 + m2p ; out = Zr^2 + Zi^2
                c2 = e_pool.tile([128, BCH], f32, tag="c2", name="c2")
                nc.vector.tensor_copy(c2[:], ps2[:])
                zr = e_pool.tile([128, BCH], f32, tag="zr", name="zr")
                nc.vector.tensor_add(zr[:], c2[:], ps1[:])
                t3 = e_pool.tile([128, BCH], f32, tag="t3", name="t3")
                nc.vector.tensor_add(t3[:], c2[:], ps3[:])
                zi = e_pool.tile([128, BCH], f32, tag="zi", name="zi")
                nc.vector.tensor_sub(zi[:], t3[:], ps1[:])
                sq_r = e_pool.tile([128, BCH], f32, tag="sqr", name="sq_r")
                nc.vector.tensor_mul(sq_r[:], zr[:], zr[:])
                sq_i = e_pool.tile([128, BCH], f32, tag="sqi", name="sq_i")
                nc.vector.tensor_mul(sq_i[:], zi[:], zi[:])
                o_sb = o_pool.tile([128, BCH], f32, name="o_sb")
                nc.vector.tensor_add(o_sb[:], sq_r[:], sq_i[:])
                nc.gpsimd.dma_start(out[psl, bsl], o_sb[:])

    nc.compile()
    _prog_cache["nc"] = nc
    return nc


def _prepare_in_maps(x_real, x_imag, patterns_real, patterns_imag, phases):
    x_real = np.ascontiguousarray(np.asarray(x_real, dtype=np.float32))
    x_imag = np.ascontiguousarray(np.asarray(x_imag, dtype=np.float32))
    patterns_real = np.asarray(patterns_real, dtype=np.float32)
    patterns_imag = np.asarray(patterns_imag, dtype=np.float32)
    ph = np.asarray(phases, dtype=np.float64)

    cos = np.cos(ph)[:, None]
    sin = np.sin(ph)[:, None]
    pr = patterns_real.astype(np.float64)
    pi = patterns_imag.astype(np.float64)
    sr = (pr * cos - pi * sin).astype(np.float32)  # [P, D]
    si = (pr * sin + pi * cos).astype(np.float32)
    srT = np.ascontiguousarray(sr.T)  # [D, P]
    siT = np.ascontiguousarray(si.T)
    ssT = srT + siT

    in_maps = []
    for c in range(NCORES):
        rows = slice(c * BC, (c + 1) * BC)
        xs = x_real[rows]
        xis = x_imag[rows]
        in_maps.append(
            {
                "xrT": np.ascontiguousarray(xs.T),
                "xiT": np.ascontiguousarray(xis.T),
                "srT": srT,
                "siT": siT,
                "ssT": ssT,
            }
        )
    return in_maps


def run(inputs, trace=False, **trace_kwargs):
    """Build + run on 8 cores. Returns (full_output, BassKernelResults)."""
    in_maps = _prepare_in_maps(**inputs)
    nc = build_program()
    try:
        res = run_bass_kernel_spmd(
            nc, in_maps, list(range(NCORES)), trace=trace, **trace_kwargs
        )
    except Exception:
        # One retry: transient NRT device errors recover on re-execution.
        import time as _time

        _time.sleep(10)
        res = run_bass_kernel_spmd(
            nc, in_maps, list(range(NCORES)), trace=trace, **trace_kwargs
        )
    full = np.concatenate(
        [np.ascontiguousarray(r["out"].T) for r in res.results], axis=0
    )
    return full, res


def kernel(x_real, x_imag, patterns_real, patterns_imag, phases):
    full, _ = run(
        {
            "x_real": x_real,
            "x_imag": x_imag,
            "patterns_real": patterns_real,
            "patterns_imag": patterns_imag,
            "phases": phases,
        }
    )
    return full
